# revision 1
# baseline (speedup 1.0000x reference)
"""Causal self-attention with RoPE on 8 NeuronCores.

Problem shapes: x [2, 2048, 2048], w_qkv [6144, 2048], w_out [2048, 2048],
H=16 heads, dh=128.

Sharding: data-parallel over batch x tensor-parallel over heads.  Core c
owns batch c//4 and heads {4g..4g+3} with g = c%4.  Each core computes a
[2048, 2048] partial of its batch's output (w_out rows for its heads);
the host sums 4 partials per batch.  Per-core DMA: x 8.4MB bf16 in,
weights 8MB bf16, out 8.4MB bf16 -- about a quarter of the all-TP/f32
layout.

All matmuls run in bf16 (1 row/cycle on PE, f32 PSUM accumulation;
measured rel err ~5e-3 vs the 2e-2 gate).  Measured cost is ~281ns per
512-wide matmul (213ns stream + ~68ns self-loading-LDWEIGHTS tax that
walrus --enable-ldw-opt=false cannot hide), so the design minimizes
matmul count and keeps every other engine off the PE critical path.
Per-core plan:

  - Q^T/K^T projection: psum[dh, tok] = w-tile (stationary) x xT (moving,
    512-token chunks); V projected into natural [tok, dh] layout by
    flipping operands (x token-slices stationary, wv moving, all 4 heads
    in one 512-wide moving pass).
  - RoPE during PSUM->SBUF copyback, 4 DVE ops per tile using
    sign-folded tables (S = [-sin; +sin], C = [cos; cos]):
    t = swap(ps) * S (2 half ops), dst = ps * C + t (2 full ops).
  - Attention per 512-token q chunk, k tiles processed in PAIRS sharing
    one [128, 1024] PSUM tile so a single exp activation covers both
    (ACT per pair 1147ns < PE per pair ~1280ns).  The attn-out/denom
    matmuls for pair p are emitted after the scores matmuls of pair p+2
    (software pipelining, 2-pair lag) so PE never waits on ACT.
  - Causality: diagonal-block matmuls (scores/attn-out/denominator) shrink
    their moving free dim to the valid q range (saves ~37% of the diagonal
    rows); the remaining triangular 128-col block of each is masked by a
    single shared 0/1 multiply (bf16 DVE).  exp runs over the full pair
    tile -- the gap regions hold stale PSUM whose exp is never read.
  - Softmax denominator accumulates ones^T x e on PE alongside attn-out;
    reciprocal via the 1-instruction approx-fast DVE op (~51 ULP),
    broadcast across partitions with gpsimd, multiplied in during the
    attn-out copyback.
  - w_out for chunk qc is emitted after attention for qc+1; projections
    for chunk c+1 interleave with attention for chunk c.
"""

import numpy as np

import concourse.bass as bass
import concourse.mybir as mybir
import concourse.tile as tile
from concourse import bacc, library_config
from concourse.bass_utils import run_bass_kernel_spmd

B, L, D, H = 2, 2048, 2048, 16
DH = D // H  # 128
NCORES = 8
NGRP = 4  # head groups (cores per batch)
HPC = H // NGRP  # 4 heads per core
ROPE_BASE = 10000.0
SCALE = 1.0 / float(np.sqrt(np.float32(DH)))

TOKC = 512  # token chunk width in the QKV projection phase
NCHUNK = L // TOKC  # 4
QC = 512  # q chunk width in the attention phase (== TOKC)
KT = L // 128  # 16 k tiles per sequence
KD = D // 128  # 16 contraction chunks for the projections

F32 = mybir.dt.float32
BF16 = mybir.dt.bfloat16
AF = mybir.ActivationFunctionType
ALU = mybir.AluOpType


def _body(nc, tc, aps, phases=("qkv", "attn", "wout")):
    xt, wq, wk, wv, wo, cc, ss, mk, out = aps
    with (
        tc.tile_pool(name="const", bufs=1) as const,
        tc.tile_pool(name="xtp", bufs=2) as xtp,
        tc.tile_pool(name="qkv", bufs=1) as qkvp,
        tc.tile_pool(name="rope", bufs=1) as ropep,
        tc.tile_pool(name="esb", bufs=4) as esbp,
        tc.tile_pool(name="esum", bufs=3) as esump,
        tc.tile_pool(name="recp", bufs=1) as recp,
        tc.tile_pool(name="bcp", bufs=2) as bcp,
        tc.tile_pool(name="attn", bufs=2) as attnp,
        tc.tile_pool(name="outp", bufs=6) as outp,
        tc.tile_pool(name="psA", bufs=2, space="PSUM") as psA,
        tc.tile_pool(name="psO", bufs=2, space="PSUM") as psO,
        tc.tile_pool(name="psS", bufs=2, space="PSUM") as psS,
    ):
        # ---- constants ----
        wq_sb = const.tile([128, KD, HPC * DH], BF16, name="wq_sb")
        wk_sb = const.tile([128, KD, HPC * DH], BF16, name="wk_sb")
        wv_sb = const.tile([128, KD, HPC * DH], BF16, name="wv_sb")
        wo_sb = const.tile([128, HPC, D], BF16, name="wo_sb")
        cc_sb = const.tile([128, L], F32, name="cc_sb")  # [cos; cos]
        ss_sb = const.tile([128, L], F32, name="ss_sb")  # [-sin; +sin]
        mk_sb = const.tile([128, 4, QC], BF16, name="mk_sb")

        def load_chunk(c):
            c0 = c * TOKC
            xtile = xtp.tile([128, KD, TOKC], BF16, name="xtile")
            src = xt[:, c0:c0 + TOKC].rearrange("(ko p) n -> p ko n", p=128)
            # two half-loads: matmuls on k<8 start as soon as half 0 lands
            nc.sync.dma_start(xtile[:, 0:KD // 2], src[:, 0:KD // 2])
            nc.sync.dma_start(xtile[:, KD // 2:], src[:, KD // 2:])
            return xtile

        # first x chunk + first weight ahead of everything else; wo last
        xtile0 = load_chunk(0) if "qkv" in phases else None
        for dst, src in ((wq_sb, wq), (cc_sb, cc), (ss_sb, ss), (mk_sb, mk),
                         (wk_sb, wk), (wv_sb, wv), (wo_sb, wo)):
            nc.sync.dma_start(dst, src)
        ones_f32 = const.tile([128, 1], F32, name="ones_f32")
        nc.vector.memset(ones_f32, 1.0)
        ones_col = const.tile([128, 1], BF16, name="ones_col")
        nc.vector.tensor_copy(ones_col, ones_f32)

        qrot = [qkvp.tile([128, L], BF16, name=f"qrot{h}") for h in range(HPC)]
        krot = [qkvp.tile([128, L], BF16, name=f"krot{h}") for h in range(HPC)]
        vnat = qkvp.tile([128, KT, HPC * DH], BF16, name="vnat")

        def proj_chunk(c):
            c0 = c * TOKC
            xtile = xtile0 if c == 0 else load_chunk(c)
            cseg = cc_sb[:, c0:c0 + TOKC]
            sseg = ss_sb[:, c0:c0 + TOKC]
            for w_sb, dsts in ((wq_sb, qrot), (wk_sb, krot)):
                for h in range(HPC):
                    ps = psA.tile([128, TOKC], F32, name="ps_proj", tag="psA")
                    for k in range(KD):
                        nc.tensor.matmul(
                            ps, w_sb[:, k, h * DH:(h + 1) * DH], xtile[:, k, :],
                            start=(k == 0), stop=(k == KD - 1),
                        )
                    if "nocopy" in phases:
                        continue
                    # RoPE copyback: dst = ps*C + swap(ps)*S
                    t = ropep.tile([128, TOKC], F32, name="rope_t")
                    a = ropep.tile([128, TOKC], F32, name="rope_a")
                    nc.vector.tensor_tensor(t[0:64], ps[64:128], sseg[0:64], ALU.mult)
                    nc.vector.tensor_tensor(t[64:128], ps[0:64], sseg[64:128], ALU.mult)
                    nc.vector.tensor_tensor(a, ps, cseg, ALU.mult)
                    nc.vector.tensor_tensor(
                        dsts[h][:, c0:c0 + TOKC], a, t, ALU.add)
            # V in natural [tok, dh] layout: x token slices stationary, all
            # 4 heads share one 512-wide moving pass
            for s in range(TOKC // 128):
                psv = psA.tile([128, HPC * DH], F32, name="ps_v", tag="psA")
                for k in range(KD):
                    nc.tensor.matmul(
                        psv, xtile[:, k, s * 128:(s + 1) * 128], wv_sb[:, k, :],
                        start=(k == 0), stop=(k == KD - 1),
                    )
                if "nocopy" in phases:
                    continue
                kti = (c0 // 128) + s
                nc.scalar.copy(vnat[:, kti, :], psv)

        def emit_wout(attn_sb, q0):
            for mt in range(QC // 128):
                t0 = q0 + mt * 128
                for ec in range(D // 512):
                    psw = psA.tile([128, 512], F32, name="ps_w", tag="psA")
                    for h in range(HPC):
                        nc.tensor.matmul(
                            psw, attn_sb[h][:, mt * 128:(mt + 1) * 128],
                            wo_sb[:, h, ec * 512:(ec + 1) * 512],
                            start=(h == 0), stop=(h == HPC - 1),
                        )
                    ob = outp.tile([128, 512], BF16, name="out_sb")
                    if (mt + ec) % 2 == 0:
                        nc.scalar.copy(ob, psw)
                    else:
                        nc.vector.tensor_copy(ob, psw)
                    eng = nc.scalar if (mt + ec) % 2 == 0 else nc.sync
                    eng.dma_start(
                        out[t0:t0 + 128, ec * 512:(ec + 1) * 512], ob)

        def attn_chunk(qc):
            q0 = qc * QC
            attn_sb = []
            for h in range(HPC):
                pso = psO.tile([128, QC], F32, name="ps_out")
                pss = psS.tile([1, QC], F32, name="ps_sum")
                nkt = (qc + 1) * (QC // 128)
                npair = nkt // 2
                epairs = [None] * npair
                esums = [None] * npair

                def emit_attnout(q):
                    e = epairs[q]
                    for half in range(2):
                        kt = 2 * q + half
                        off = 128 * max(kt - 4 * qc, 0)  # causal shrink
                        eh = e[:, half * QC + off:(half + 1) * QC]
                        nc.tensor.matmul(
                            pso[:, off:QC], vnat[:, kt, h * DH:(h + 1) * DH],
                            eh, start=(kt == 0), stop=(kt == nkt - 1))
                    if "nosum" not in phases:
                        # one denominator matmul per PAIR over the
                        # DVE-presummed e0+e1 (halves the ones-matmul count)
                        off0 = 128 * max(2 * q - 4 * qc, 0)
                        nc.tensor.matmul(
                            pss[:, off0:QC], ones_col, esums[q][:, off0:QC],
                            start=(q == 0), stop=(q == npair - 1))

                for p in range(npair + 2):
                    if p < npair:
                        pp = psA.tile([128, 2 * QC], F32, name="ps_sc", tag="psA")
                        for half in range(2):
                            kt = 2 * p + half
                            off = 128 * max(kt - 4 * qc, 0)  # causal shrink
                            nc.tensor.matmul(
                                pp[:, half * QC + off:(half + 1) * QC],
                                krot[h][:, kt * 128:(kt + 1) * 128],
                                qrot[h][:, q0 + off:q0 + QC],
                                start=True, stop=True,
                            )
                        e = esbp.tile([128, 2 * QC], BF16, name="e_sb")
                        if "noexp" in phases:
                            nc.scalar.copy(e, pp)
                        else:
                            nc.scalar.activation(e, pp, AF.Exp, scale=SCALE)
                        if "nomask" not in phases:
                            for half in range(2):
                                if 2 * p + half - 4 * qc >= 0:
                                    off = (half * QC
                                           + 128 * (2 * p + half - 4 * qc))
                                    eh = e[:, off:off + 128]
                                    nc.vector.tensor_tensor(
                                        eh, eh, mk_sb[:, 0, 0:128], ALU.mult)
                        epairs[p] = e
                        if "nosum" not in phases:
                            es = esump.tile([128, QC], BF16, name="esum")
                            d0 = 2 * p - 4 * qc
                            if d0 >= 0:  # diagonal pair: halves differ by 128
                                o = 128 * d0
                                nc.vector.tensor_copy(
                                    es[:, o:o + 128], e[:, o:o + 128])
                                nc.vector.tensor_tensor(
                                    es[:, o + 128:QC], e[:, o + 128:QC],
                                    e[:, QC + o + 128:2 * QC], ALU.add)
                            else:
                                nc.vector.tensor_tensor(
                                    es[:, 0:QC], e[:, 0:QC], e[:, QC:2 * QC],
                                    ALU.add)
                            esums[p] = es
                    if p >= 2:
                        emit_attnout(p - 2)

                att = attnp.tile([128, QC], BF16, name=f"att{h}")
                if "nosum" in phases:
                    nc.vector.tensor_copy(att, pso)
                else:
                    rec = recp.tile([1, QC], F32, name="recip")
                    nc.vector.reciprocal_approx_fast(rec, pss)
                    bc = bcp.tile([128, QC], F32, name="bc_sb")
                    nc.gpsimd.partition_broadcast(bc, rec)
                    nc.vector.tensor_tensor(att, pso, bc, ALU.mult)
                attn_sb.append(att)
            return attn_sb

        pend = None
        for c in range(NCHUNK):
            if "qkv" in phases:
                proj_chunk(c)
            if "attn" in phases:
                attn_sb = attn_chunk(c)
                if pend is not None and "wout" in phases:
                    emit_wout(*pend)
                pend = (attn_sb, c * QC)
        if pend is not None and "wout" in phases:
            emit_wout(*pend)


def build_kernel(timing=False, loop_n=0, phases=("qkv", "attn", "wout")):
    nc = bacc.Bacc(
        "TRN2",
        target_bir_lowering=False,
        debug=False,
        enable_asserts=False,
        num_devices=NCORES,
    )
    xt = nc.dram_tensor("xt", [D, L], BF16, kind="ExternalInput").ap()
    wq = nc.dram_tensor("wq", [128, KD, HPC * DH], BF16, kind="ExternalInput").ap()
    wk = nc.dram_tensor("wk", [128, KD, HPC * DH], BF16, kind="ExternalInput").ap()
    wv = nc.dram_tensor("wv", [128, KD, HPC * DH], BF16, kind="ExternalInput").ap()
    wo = nc.dram_tensor("wo", [128, HPC, D], BF16, kind="ExternalInput").ap()
    cc = nc.dram_tensor("cc", [128, L], F32, kind="ExternalInput").ap()
    ss = nc.dram_tensor("ss", [128, L], F32, kind="ExternalInput").ap()
    mk = nc.dram_tensor("mk", [128, 4, QC], BF16, kind="ExternalInput").ap()
    out_kind = "Internal" if timing else "ExternalOutput"
    out = nc.dram_tensor("out", [L, D], BF16, kind=out_kind).ap()
    done = None
    if timing:
        done = nc.dram_tensor("done", [1, 4], BF16, kind="ExternalOutput").ap()

    nc.gpsimd.load_library(library_config.attn)
    aps = (xt, wq, wk, wv, wo, cc, ss, mk, out)
    with tile.TileContext(nc) as tc:
        if loop_n:
            with tc.For_i(0, loop_n, 1):
                _body(nc, tc, aps, phases)
        else:
            _body(nc, tc, aps, phases)
        if timing:
            # tiny output so the executable has an ExternalOutput; depends on
            # one real out tile via a DRAM->DRAM DMA of the last row.
            nc.sync.dma_start(done, out[L - 1:L, 0:4])
    nc.compile()
    return nc


def _rope_tables():
    inv_freq = (1.0 / (ROPE_BASE ** (np.arange(0, DH, 2, dtype=np.float32) / DH))
                ).astype(np.float32)
    freqs = (np.arange(L, dtype=np.float32)[:, None] * inv_freq[None, :]
             ).astype(np.float32)  # [L, 64]
    cos_t = np.cos(freqs).astype(np.float32).T  # [64, L]
    sin_t = np.sin(freqs).astype(np.float32).T
    cc = np.concatenate([cos_t, cos_t], axis=0)  # [128, L]
    ss = np.concatenate([-sin_t, sin_t], axis=0)
    return np.ascontiguousarray(cc), np.ascontiguousarray(ss)


def _host_inputs(x, w_qkv, w_out):
    bf16 = mybir.dt.np(BF16)
    cc, ss = _rope_tables()
    p = np.arange(128)[:, None]
    f = np.arange(QC)[None, :]
    mk = np.stack(
        [((bi * 128 + p) <= f).astype(np.float32) for bi in range(4)], axis=1
    ).astype(bf16)  # [128, 4, 512]
    mk = np.ascontiguousarray(mk)

    def wtile(wT):  # [D, M] f32 -> [128, D//128, M] bf16
        return np.ascontiguousarray(
            wT.reshape(KD, 128, wT.shape[1]).transpose(1, 0, 2)).astype(bf16)

    xts = [
        np.ascontiguousarray(x[b].T).astype(bf16)  # [D, L]
        for b in range(B)
    ]
    in_maps = []
    for c in range(NCORES):
        b, g = divmod(c, NGRP)
        r0 = g * HPC * DH
        r1 = r0 + HPC * DH
        wq_c = wtile(np.ascontiguousarray(w_qkv[r0:r1, :].T))
        wk_c = wtile(np.ascontiguousarray(w_qkv[D + r0:D + r1, :].T))
        wv_c = wtile(np.ascontiguousarray(w_qkv[2 * D + r0:2 * D + r1, :].T))
        wo_c = np.ascontiguousarray(
            w_out[:, r0:r1].T.reshape(HPC, 128, D).transpose(1, 0, 2)
        ).astype(bf16)
        in_maps.append({
            "xt": xts[b], "wq": wq_c, "wk": wk_c, "wv": wv_c, "wo": wo_c,
            "cc": cc, "ss": ss, "mk": mk,
        })
    return in_maps


_NC_CACHE = []


def _get_nc():
    if not _NC_CACHE:
        _NC_CACHE.append(build_kernel())
    return _NC_CACHE[0]


def kernel(x, w_qkv, w_out):
    x = np.asarray(x, dtype=np.float32)
    w_qkv = np.asarray(w_qkv, dtype=np.float32)
    w_out = np.asarray(w_out, dtype=np.float32)
    nc = _get_nc()
    in_maps = _host_inputs(x, w_qkv, w_out)
    res = run_bass_kernel_spmd(nc, in_maps, core_ids=list(range(NCORES)))
    out = np.zeros((B, L, D), dtype=np.float32)
    for c in range(NCORES):
        out[c // NGRP] += res.results[c]["out"].astype(np.float32)
    return out



# revision 35
# speedup vs baseline: 17.1896x; 17.1896x over previous
"""Causal self-attention with RoPE on 8 NeuronCores.

Problem shapes: x [2, 2048, 2048], w_qkv [6144, 2048], w_out [2048, 2048],
H=16 heads, dh=128.

Sharding: data-parallel over batch x tensor-parallel over heads.  Core c
owns batch c//4 and heads {4g..4g+3} with g = c%4.  Each core computes a
[2048, 2048] partial of its batch's output (w_out rows for its heads);
the host sums 4 partials per batch.  Per-core DMA: x 8.4MB bf16 in,
weights 8MB bf16, out 8.4MB bf16 -- about a quarter of the all-TP/f32
layout.

All matmuls run in bf16 (1 row/cycle on PE, f32 PSUM accumulation;
measured rel err ~5e-3 vs the 2e-2 gate).  The PE stream floor for this
decomposition is ~285us/core-iter; the design keeps the matmul pipe >94%
busy by keeping every other engine (and the DMA rings) off its critical
path.  Per-core plan:

  - Q^T/K^T projection: psum[dh, tok] = w-tile (stationary) x xT (moving,
    512-token chunks); V projected into natural [tok, dh] layout by
    flipping operands (x token-slices stationary, wv moving, all 4 heads
    in one 512-wide moving pass).
  - RoPE during PSUM->SBUF copyback, 4 DVE ops per tile using
    sign-folded tables (S = [-sin; +sin], C = [cos; cos], bf16):
    t = swap(ps) * S (2 half ops, PSUM-read partition shift), dst =
    ps * C + t; t/a in bf16 so the add takes the DVE 2x 16-bit path.
  - Attention per 512-token q chunk, k tiles processed in PAIRS sharing
    one [128, 1024] PSUM tile so a single exp activation covers both.
    Attn-out for pair p is emitted after the scores matmuls of pair p+3
    (3-tick lag covers the exp->mask DVE chain on the in-order PE).
  - Causality: diagonal-block matmuls (scores/attn-out) shrink their
    moving free dim to the valid q range; the remaining triangular
    128-col block is masked by a 0/1 multiply (bf16 DVE).  exp runs over
    the full pair tile -- gap regions hold stale PSUM that is never read.
  - Softmax denominator: pair sums e0+e1 (DVE bf16) are folded once more
    into QUAD sums, and a single ones^T x esq matmul per quad accumulates
    the denominator on PE (quarter the ones-matmul stream of per-pair),
    emitted 3 ticks after its esq so the PE never waits on the DVE chain.
    Reciprocal via the approx-fast DVE op (~51 ULP), broadcast across
    partitions with gpsimd, multiplied in during the attn-out copyback.
  - w_out for chunk qc is emitted after attention for qc+1; output rows
    are assembled into [128, 2048] tiles so stores move 4KB-contiguous
    rows (4KB DMA packets ~3x the store bandwidth of 1KB ones).
  - DMA discipline: all loads issue from the sync engine (ring Q1), all
    stores from the scalar engine (Q10), so loads never queue behind
    stores.  Host-side x layout is [128, chunk, kd, tok] so each chunk
    load is 16KB-contiguous per partition.  Load order minimizes
    time-to-first-matmul (x0 half, wq half first, wo last).
  - The timing build unrolls the body 8x inside tc.For_i: the loop
    back-edge is an all-engine barrier + semaphore reset, so unrolling
    amortizes it and restores cross-body DMA/compute overlap
    (staggered_reset softens the remaining barrier).
"""

import numpy as np

import concourse.bass as bass
import concourse.mybir as mybir
import concourse.tile as tile
from concourse import bacc, library_config
from concourse.bass_utils import run_bass_kernel_spmd

B, L, D, H = 2, 2048, 2048, 16
DH = D // H  # 128
NCORES = 8
NGRP = 4  # head groups (cores per batch)
HPC = H // NGRP  # 4 heads per core
ROPE_BASE = 10000.0
SCALE = 1.0 / float(np.sqrt(np.float32(DH)))

TOKC = 512  # token chunk width in the QKV projection phase
NCHUNK = L // TOKC  # 4
QC = 512  # q chunk width in the attention phase (== TOKC)
KT = L // 128  # 16 k tiles per sequence
KD = D // 128  # 16 contraction chunks for the projections

F32 = mybir.dt.float32
BF16 = mybir.dt.bfloat16
AF = mybir.ActivationFunctionType
ALU = mybir.AluOpType


def _make_pools(tc, stk):
    pool = lambda name, bufs, **kw: stk.enter_context(  # noqa: E731
        tc.tile_pool(name=name, bufs=bufs, **kw))
    return {
        "const": pool("const", 1),
        "xtp": pool("xtp", 2),
        "qkvp": pool("qkv", 1),
        "ropep": pool("rope", 1),
        "esbp": pool("esb", 5),
        "esump": pool("esum", 3),
        "esqp": pool("esq", 3),
        "recp": pool("recp", 1),
        "bcp": pool("bcp", 2),
        "attnp": pool("attn", 2),
        "outp": pool("outp", 3),
        "psA": pool("psA", 2, space="PSUM"),
        "psO": pool("psO", 2, space="PSUM"),
        "psS": pool("psS", 2, space="PSUM"),
    }


def _body(nc, tc, aps, P, phases=("qkv", "attn", "wout")):
    xt, wq, wk, wv, wo, cc, ss, mk, out = aps
    const, xtp, qkvp, ropep = P["const"], P["xtp"], P["qkvp"], P["ropep"]
    esbp, esump, recp, bcp = P["esbp"], P["esump"], P["recp"], P["bcp"]
    esqp = P["esqp"]
    attnp, outp = P["attnp"], P["outp"]
    psA, psO, psS = P["psA"], P["psO"], P["psS"]
    if True:
        # ---- constants ----
        wq_sb = const.tile([128, KD, HPC * DH], BF16, name="wq_sb")
        wk_sb = const.tile([128, KD, HPC * DH], BF16, name="wk_sb")
        wv_sb = const.tile([128, KD, HPC * DH], BF16, name="wv_sb")
        wo_sb = const.tile([128, HPC, D], BF16, name="wo_sb")
        cc_sb = const.tile([128, NCHUNK, TOKC], BF16, name="cc_sb")  # [cos; cos]
        ss_sb = const.tile([128, NCHUNK, TOKC], BF16, name="ss_sb")  # [-sin; +sin]
        mk_sb = const.tile([128, 128], BF16, name="mk_sb")

        def load_chunk(c):
            xtile = xtp.tile([128, KD, TOKC], BF16, name="xtile")
            src = xt[:, c]  # [128, KD, TOKC], 16KB contiguous per partition
            # two half-loads: matmuls on k<8 start as soon as half 0 lands
            nc.sync.dma_start(xtile[:, 0:KD // 2], src[0:128, 0:KD // 2])
            nc.sync.dma_start(xtile[:, KD // 2:], src[0:128, KD // 2:])
            # RoPE tables for this chunk (contiguous 2KB/partition slices)
            nc.sync.dma_start(cc_sb[:, c], cc[0:128, c * TOKC:(c + 1) * TOKC])
            nc.sync.dma_start(ss_sb[:, c], ss[0:128, c * TOKC:(c + 1) * TOKC])
            return xtile

        # Load order minimizes time-to-first-matmul: the first projection
        # matmuls need only x chunk-0 half a and the wq k<8 half; everything
        # else lands behind them in consumption order (wo last).
        xtile0 = None
        if "qkv" in phases:
            xtile0 = xtp.tile([128, KD, TOKC], BF16, name="xtile")
            nc.sync.dma_start(xtile0[:, 0:KD // 2], xt[:, 0, 0:KD // 2])
        KH = KD // 2
        nc.sync.dma_start(wq_sb[:, 0:KH], wq[:, 0:KH])
        if "qkv" in phases:
            nc.sync.dma_start(xtile0[:, KH:], xt[:, 0, KH:])
            nc.sync.dma_start(cc_sb[:, 0], cc[0:128, 0:TOKC])
            nc.sync.dma_start(ss_sb[:, 0], ss[0:128, 0:TOKC])
        nc.sync.dma_start(wq_sb[:, KH:], wq[:, KH:])
        nc.sync.dma_start(wk_sb[:, 0:KH], wk[:, 0:KH])
        nc.sync.dma_start(wk_sb[:, KH:], wk[:, KH:])
        nc.sync.dma_start(wv_sb[:, 0:KH], wv[:, 0:KH])
        nc.sync.dma_start(wv_sb[:, KH:], wv[:, KH:])
        nc.sync.dma_start(mk_sb, mk)
        nc.sync.dma_start(wo_sb, wo)
        ones_f32 = const.tile([128, 1], F32, name="ones_f32")
        nc.vector.memset(ones_f32, 1.0)
        ones_col = const.tile([128, 1], BF16, name="ones_col")
        nc.vector.tensor_copy(ones_col, ones_f32)

        qrot = [qkvp.tile([128, L], BF16, name=f"qrot{h}") for h in range(HPC)]
        krot = [qkvp.tile([128, L], BF16, name=f"krot{h}") for h in range(HPC)]
        vnat = qkvp.tile([128, KT, HPC * DH], BF16, name="vnat")

        def proj_chunk(c):
            c0 = c * TOKC
            xtile = xtile0 if c == 0 else load_chunk(c)
            cseg = cc_sb[:, c]
            sseg = ss_sb[:, c]
            for w_sb, dsts in ((wq_sb, qrot), (wk_sb, krot)):
                for h in range(HPC):
                    ps = psA.tile([128, TOKC], F32, name="ps_proj", tag="psA")
                    for k in range(KD):
                        nc.tensor.matmul(
                            ps, w_sb[:, k, h * DH:(h + 1) * DH], xtile[:, k, :],
                            start=(k == 0), stop=(k == KD - 1),
                        )
                    if "nocopy" in phases:
                        continue
                    # RoPE copyback: dst = ps*C + swap(ps)*S.  The swap reads
                    # must keep ps in PSUM (partition-shifted SBUF-SBUF reads
                    # are illegal); t/a in bf16 makes the final add take the
                    # DVE double-pumped 16-bit path.
                    t = ropep.tile([128, TOKC], BF16, name="rope_t")
                    a = ropep.tile([128, TOKC], BF16, name="rope_a")
                    nc.vector.tensor_tensor(t[0:64], ps[64:128], sseg[0:64], ALU.mult)
                    nc.vector.tensor_tensor(t[64:128], ps[0:64], sseg[64:128], ALU.mult)
                    nc.vector.tensor_tensor(a, ps, cseg, ALU.mult)
                    nc.vector.tensor_tensor(
                        dsts[h][:, c0:c0 + TOKC], a, t, ALU.add)
            # V in natural [tok, dh] layout: x token slices stationary, all
            # 4 heads share one 512-wide moving pass
            for s in range(TOKC // 128):
                psv = psA.tile([128, HPC * DH], F32, name="ps_v", tag="psA")
                for k in range(KD):
                    nc.tensor.matmul(
                        psv, xtile[:, k, s * 128:(s + 1) * 128], wv_sb[:, k, :],
                        start=(k == 0), stop=(k == KD - 1),
                    )
                if "nocopy" in phases:
                    continue
                kti = (c0 // 128) + s
                nc.scalar.copy(vnat[:, kti, :], psv)

        def emit_wout(attn_sb, q0):
            for mt in range(QC // 128):
                t0 = q0 + mt * 128
                # one wide [128, D] output tile per 128-token row: the store
                # then moves 4KB-contiguous rows (4KB DMA packets) instead of
                # 1KB ones, and all stores ride the scalar ring so the sync
                # ring carries only loads
                ob = outp.tile([128, D], BF16, name="out_sb")
                for ec in range(D // 512):
                    psw = psA.tile([128, 512], F32, name="ps_w", tag="psA")
                    for h in range(HPC):
                        nc.tensor.matmul(
                            psw, attn_sb[h][:, mt * 128:(mt + 1) * 128],
                            wo_sb[:, h, ec * 512:(ec + 1) * 512],
                            start=(h == 0), stop=(h == HPC - 1),
                        )
                    oseg = ob[:, ec * 512:(ec + 1) * 512]
                    # copybacks alternate scalar/vector, but the last two psw
                    # tiles of the body land on scalar: the next body's first
                    # proj matmuls WAR on these psA slots, and the vector
                    # queue is still busy with attention tail work
                    if (mt + ec) % 2 == 0 or (mt == 3 and ec == 2):
                        nc.scalar.copy(oseg, psw)
                    else:
                        nc.vector.tensor_copy(oseg, psw)
                nc.scalar.dma_start(out[t0:t0 + 128, :], ob)

        def attn_chunk(qc):
            q0 = qc * QC
            attn_sb = []
            for h in range(HPC):
                pso = psO.tile([128, QC], F32, name="ps_out")
                pss = psS.tile([1, QC], F32, name="ps_sum")
                nkt = (qc + 1) * (QC // 128)
                npair = nkt // 2
                nquad = npair // 2
                epairs = [None] * npair
                esums = [None] * npair
                esqs = [None] * nquad

                def emit_attnout(q):
                    e = epairs[q]
                    for half in range(2):
                        kt = 2 * q + half
                        off = 128 * max(kt - 4 * qc, 0)  # causal shrink
                        eh = e[:, half * QC + off:(half + 1) * QC]
                        nc.tensor.matmul(
                            pso[:, off:QC], vnat[:, kt, h * DH:(h + 1) * DH],
                            eh, start=(kt == 0), stop=(kt == nkt - 1))

                for p in range(npair + 4):
                    if p < npair:
                        pp = psA.tile([128, 2 * QC], F32, name="ps_sc", tag="psA")
                        for half in range(2):
                            kt = 2 * p + half
                            off = 128 * max(kt - 4 * qc, 0)  # causal shrink
                            nc.tensor.matmul(
                                pp[:, half * QC + off:(half + 1) * QC],
                                krot[h][:, kt * 128:(kt + 1) * 128],
                                qrot[h][:, q0 + off:q0 + QC],
                                start=True, stop=True,
                            )
                        e = esbp.tile([128, 2 * QC], BF16, name="e_sb")
                        if "noexp" in phases:
                            nc.scalar.copy(e, pp)
                        else:
                            nc.scalar.activation(e, pp, AF.Exp, scale=SCALE)
                        if "nomask" not in phases:
                            for half in range(2):
                                if 2 * p + half - 4 * qc >= 0:
                                    off = (half * QC
                                           + 128 * (2 * p + half - 4 * qc))
                                    eh = e[:, off:off + 128]
                                    nc.vector.tensor_tensor(
                                        eh, eh, mk_sb, ALU.mult)
                        epairs[p] = e
                        if "nosum" not in phases:
                            es = esump.tile([128, QC], BF16, name="esum")
                            d0 = 2 * p - 4 * qc
                            if d0 >= 0:  # diagonal pair: halves differ by 128
                                o = 128 * d0
                                nc.vector.tensor_copy(
                                    es[:, o:o + 128], e[:, o:o + 128])
                                nc.vector.tensor_tensor(
                                    es[:, o + 128:QC], e[:, o + 128:QC],
                                    e[:, QC + o + 128:2 * QC], ALU.add)
                            else:
                                nc.vector.tensor_tensor(
                                    es[:, 0:QC], e[:, 0:QC], e[:, QC:2 * QC],
                                    ALU.add)
                            esums[p] = es
                            if p % 2 == 1:
                                # quad presum: fold the two pair-sums into one
                                # tile so the ones-matmul runs once per quad.
                                # The last quad's high pair is diagonal and
                                # valid only from column 256 (its first k-tile
                                # starts 2 blocks past the quad's base).
                                esq = esqp.tile([128, QC], BF16, name="esq")
                                lo, hi = esums[p - 1], esums[p]
                                if p == npair - 1:
                                    nc.vector.tensor_copy(
                                        esq[:, 0:256], lo[:, 0:256])
                                    nc.vector.tensor_tensor(
                                        esq[:, 256:QC], lo[:, 256:QC],
                                        hi[:, 256:QC], ALU.add)
                                else:
                                    nc.vector.tensor_tensor(
                                        esq, lo, hi, ALU.add)
                                esqs[p // 2] = esq
                    # attnout lags 3 ticks behind scores (covers the
                    # exp->mask DVE chain); the quad denominator matmul lags
                    # its esq presum by 3 ticks too, so the in-order PE never
                    # waits on the exp->mask->es->esq chain
                    if 3 <= p < npair + 3:
                        emit_attnout(p - 3)
                    if "nosum" not in phases and p >= 4 and p % 2 == 0:
                        j = (p - 4) // 2
                        if j < nquad:
                            nc.tensor.matmul(
                                pss, ones_col, esqs[j],
                                start=(j == 0), stop=(j == nquad - 1))

                att = attnp.tile([128, QC], BF16, name=f"att{h}")
                if "nosum" in phases:
                    nc.vector.tensor_copy(att, pso)
                else:
                    rec = recp.tile([1, QC], F32, name="recip")
                    nc.vector.reciprocal_approx_fast(rec, pss)
                    bc = bcp.tile([128, QC], F32, name="bc_sb")
                    nc.gpsimd.partition_broadcast(bc, rec)
                    nc.vector.tensor_tensor(att, pso, bc, ALU.mult)
                attn_sb.append(att)
            return attn_sb

        pend = None
        for c in range(NCHUNK):
            if "qkv" in phases:
                proj_chunk(c)
            if "attn" in phases:
                attn_sb = attn_chunk(c)
                if pend is not None and "wout" in phases:
                    emit_wout(*pend)
                pend = (attn_sb, c * QC)
        if pend is not None and "wout" in phases:
            emit_wout(*pend)


def build_kernel(timing=False, loop_n=0, phases=("qkv", "attn", "wout"),
                 unroll=8):
    nc = bacc.Bacc(
        "TRN2",
        target_bir_lowering=False,
        debug=False,
        enable_asserts=False,
        num_devices=NCORES,
    )
    xt = nc.dram_tensor(
        "xt", [128, NCHUNK, KD, TOKC], BF16, kind="ExternalInput").ap()
    wq = nc.dram_tensor("wq", [128, KD, HPC * DH], BF16, kind="ExternalInput").ap()
    wk = nc.dram_tensor("wk", [128, KD, HPC * DH], BF16, kind="ExternalInput").ap()
    wv = nc.dram_tensor("wv", [128, KD, HPC * DH], BF16, kind="ExternalInput").ap()
    wo = nc.dram_tensor("wo", [128, HPC, D], BF16, kind="ExternalInput").ap()
    cc = nc.dram_tensor("cc", [128, L], BF16, kind="ExternalInput").ap()
    ss = nc.dram_tensor("ss", [128, L], BF16, kind="ExternalInput").ap()
    mk = nc.dram_tensor("mk", [128, 128], BF16, kind="ExternalInput").ap()
    out_kind = "Internal" if timing else "ExternalOutput"
    out = nc.dram_tensor("out", [L, D], BF16, kind=out_kind).ap()
    done = None
    if timing:
        done = nc.dram_tensor("done", [1, 4], BF16, kind="ExternalOutput").ap()

    nc.gpsimd.load_library(library_config.attn)
    aps = (xt, wq, wk, wv, wo, cc, ss, mk, out)
    from contextlib import ExitStack

    with tile.TileContext(nc) as tc, ExitStack() as stk:
        pools = _make_pools(tc, stk)
        if loop_n:
            # Unroll: For_i puts an all-engine barrier + semaphore reset at
            # every loop back-edge, which forbids cross-iteration DMA/compute
            # overlap.  Unrolling the body amortizes that barrier and lets
            # consecutive bodies overlap via normal tile dependency tracking.
            u = unroll if loop_n % unroll == 0 else 1
            with tc.For_i(0, loop_n // u, 1, staggered_reset=True):
                for _ in range(u):
                    _body(nc, tc, aps, pools, phases)
        else:
            _body(nc, tc, aps, pools, phases)
        if timing:
            # tiny output so the executable has an ExternalOutput; depends on
            # one real out tile via a DRAM->DRAM DMA of the last row.
            nc.sync.dma_start(done, out[L - 1:L, 0:4])
    nc.compile()
    return nc


def _rope_tables():
    inv_freq = (1.0 / (ROPE_BASE ** (np.arange(0, DH, 2, dtype=np.float32) / DH))
                ).astype(np.float32)
    freqs = (np.arange(L, dtype=np.float32)[:, None] * inv_freq[None, :]
             ).astype(np.float32)  # [L, 64]
    cos_t = np.cos(freqs).astype(np.float32).T  # [64, L]
    sin_t = np.sin(freqs).astype(np.float32).T
    cc = np.concatenate([cos_t, cos_t], axis=0)  # [128, L]
    ss = np.concatenate([-sin_t, sin_t], axis=0)
    return np.ascontiguousarray(cc), np.ascontiguousarray(ss)


def _host_inputs(x, w_qkv, w_out):
    bf16 = mybir.dt.np(BF16)
    cc, ss = _rope_tables()
    p = np.arange(128)[:, None]
    f = np.arange(128)[None, :]
    mk = np.ascontiguousarray((p <= f).astype(np.float32).astype(bf16))

    def wtile(wT):  # [D, M] f32 -> [128, D//128, M] bf16
        return np.ascontiguousarray(
            wT.reshape(KD, 128, wT.shape[1]).transpose(1, 0, 2)).astype(bf16)

    xts = [
        # [128, NCHUNK, KD, TOKC]: per partition, each chunk is one 16KB
        # contiguous block so the chunk DMA runs at full HBM bandwidth
        np.ascontiguousarray(
            x[b].T.reshape(KD, 128, NCHUNK, TOKC).transpose(1, 2, 0, 3)
        ).astype(bf16)
        for b in range(B)
    ]
    in_maps = []
    for c in range(NCORES):
        b, g = divmod(c, NGRP)
        r0 = g * HPC * DH
        r1 = r0 + HPC * DH
        wq_c = wtile(np.ascontiguousarray(w_qkv[r0:r1, :].T))
        wk_c = wtile(np.ascontiguousarray(w_qkv[D + r0:D + r1, :].T))
        wv_c = wtile(np.ascontiguousarray(w_qkv[2 * D + r0:2 * D + r1, :].T))
        wo_c = np.ascontiguousarray(
            w_out[:, r0:r1].T.reshape(HPC, 128, D).transpose(1, 0, 2)
        ).astype(bf16)
        in_maps.append({
            "xt": xts[b], "wq": wq_c, "wk": wk_c, "wv": wv_c, "wo": wo_c,
            "cc": cc.astype(bf16), "ss": ss.astype(bf16), "mk": mk,
        })
    return in_maps


_NC_CACHE = []


def _get_nc():
    if not _NC_CACHE:
        _NC_CACHE.append(build_kernel())
    return _NC_CACHE[0]


def kernel(x, w_qkv, w_out):
    x = np.asarray(x, dtype=np.float32)
    w_qkv = np.asarray(w_qkv, dtype=np.float32)
    w_out = np.asarray(w_out, dtype=np.float32)
    nc = _get_nc()
    in_maps = _host_inputs(x, w_qkv, w_out)
    res = run_bass_kernel_spmd(nc, in_maps, core_ids=list(range(NCORES)))
    out = np.zeros((B, L, D), dtype=np.float32)
    for c in range(NCORES):
        out[c // NGRP] += res.results[c]["out"].astype(np.float32)
    return out



# revision 47
# speedup vs baseline: 18.5972x; 1.0819x over previous
"""Causal self-attention with RoPE on 8 NeuronCores.

Problem shapes: x [2, 2048, 2048], w_qkv [6144, 2048], w_out [2048, 2048],
H=16 heads, dh=128.

Sharding: data-parallel over batch x tensor-parallel over heads.  Core c
owns batch c//4 and heads {4g..4g+3} with g = c%4.  Each core computes a
[2048, 2048] partial of its batch's output (w_out rows for its heads);
the host sums 4 partials per batch.  Per-core DMA: x 8.4MB bf16 in,
weights 8MB bf16, out 8.4MB bf16 -- about a quarter of the all-TP/f32
layout.

All matmuls run in bf16 (1 row/cycle on PE, f32 PSUM accumulation;
measured rel err ~5e-3 vs the 2e-2 gate).  The PE stream floor for this
decomposition is ~285us/core-iter; the design keeps the matmul pipe >94%
busy by keeping every other engine (and the DMA rings) off its critical
path.  Per-core plan:

  - Q^T/K^T projection: psum[dh, tok] = w-tile (stationary) x xT (moving,
    512-token chunks); V projected into natural [tok, dh] layout by
    flipping operands (x token-slices stationary, wv moving, all 4 heads
    in one 512-wide moving pass).
  - RoPE during PSUM->SBUF copyback, 4 DVE ops per tile using
    sign-folded tables (S = [-sin; +sin], C = [cos; cos], bf16):
    t = swap(ps) * S (2 half ops, PSUM-read partition shift), dst =
    ps * C + t; t/a in bf16 so the add takes the DVE 2x 16-bit path.
  - Attention per 512-token q chunk, k tiles processed in PAIRS sharing
    one [128, 1024] PSUM tile so a single exp activation covers both.
    Attn-out for pair p is emitted after the scores matmuls of pair p+3
    (3-tick lag covers the exp->mask DVE chain on the in-order PE).
  - Causality: diagonal-block matmuls (scores/attn-out) shrink their
    moving free dim to the valid q range; the remaining triangular
    128-col block is masked by a 0/1 multiply (bf16 DVE).  exp runs over
    the full pair tile -- gap regions hold stale PSUM that is never read.
  - Softmax denominator: pair sums e0+e1 (DVE bf16) are folded once more
    into QUAD sums, and a single ones^T x esq matmul per quad accumulates
    the denominator on PE (quarter the ones-matmul stream of per-pair),
    emitted 3 ticks after its esq so the PE never waits on the DVE chain.
    Reciprocal via the approx-fast DVE op (~51 ULP), broadcast across
    partitions with gpsimd, multiplied in during the attn-out copyback.
  - w_out is INTERLEAVED into the next chunk's attention tick stream
    (one 4-matmul psw group per tick, own 1-bank PSUM pool, copybacks on
    vector, stores on the sync ring): within an attention chunk the ACT
    exp throughput (~1.15us/pair) exceeds the PE's own tick work
    (~0.9us/pair), so without filler the in-order PE starves behind the
    exp chain -- the interleave cut measured PE idle from ~21 to ~5
    us/iter (PE >97% busy).  The final chunk's w_out crosses into the
    NEXT body's first attention chunk (flushed as a block only before
    the loop barrier).  Output rows are assembled into [128, 2048]
    tiles so stores move 4KB-contiguous rows.
  - DMA discipline: all loads issue from the sync engine (ring Q1), all
    stores from the scalar engine (Q10), so loads never queue behind
    stores.  Host-side x layout is [128, chunk, kd, tok] so each chunk
    load is 16KB-contiguous per partition.  Load order minimizes
    time-to-first-matmul (x0 half, wq half first, wo last).
  - The timing build unrolls the body 8x inside tc.For_i: the loop
    back-edge is an all-engine barrier + semaphore reset, so unrolling
    amortizes it and restores cross-body DMA/compute overlap
    (staggered_reset softens the remaining barrier).
"""

import numpy as np

import concourse.bass as bass
import concourse.mybir as mybir
import concourse.tile as tile
from concourse import bacc, library_config
from concourse.bass_utils import run_bass_kernel_spmd

B, L, D, H = 2, 2048, 2048, 16
DH = D // H  # 128
NCORES = 8
NGRP = 4  # head groups (cores per batch)
HPC = H // NGRP  # 4 heads per core
ROPE_BASE = 10000.0
SCALE = 1.0 / float(np.sqrt(np.float32(DH)))

TOKC = 512  # token chunk width in the QKV projection phase
NCHUNK = L // TOKC  # 4
QC = 512  # q chunk width in the attention phase (== TOKC)
KT = L // 128  # 16 k tiles per sequence
KD = D // 128  # 16 contraction chunks for the projections

F32 = mybir.dt.float32
BF16 = mybir.dt.bfloat16
AF = mybir.ActivationFunctionType
ALU = mybir.AluOpType


def _make_pools(tc, stk):
    pool = lambda name, bufs, **kw: stk.enter_context(  # noqa: E731
        tc.tile_pool(name=name, bufs=bufs, **kw))
    return {
        "const": pool("const", 1),
        "xtp": pool("xtp", 2),
        "qkvp": pool("qkv", 1),
        "ropep": pool("rope", 1),
        "esbp": pool("esb", 5),
        "esump": pool("esum", 3),
        "esqp": pool("esq", 3),
        "recp": pool("recp", 1),
        "bcp": pool("bcp", 2),
        "attnp": pool("attn", 2),
        "outp": pool("outp", 4),
        "psA": pool("psA", 2, space="PSUM"),
        "psO": pool("psO", 2, space="PSUM"),
        "psS": pool("psS", 1, space="PSUM"),
        "psW": pool("psW", 1, space="PSUM"),
    }


def _body(nc, tc, aps, P, phases=("qkv", "attn", "wout"), pend_in=None,
          flush_tail=True):
    xt, wq, wk, wv, wo, cc, ss, mk, out = aps
    const, xtp, qkvp, ropep = P["const"], P["xtp"], P["qkvp"], P["ropep"]
    esbp, esump, recp, bcp = P["esbp"], P["esump"], P["recp"], P["bcp"]
    esqp = P["esqp"]
    attnp, outp = P["attnp"], P["outp"]
    psA, psO, psS, psW = P["psA"], P["psO"], P["psS"], P["psW"]
    if True:
        # ---- constants ----
        wq_sb = const.tile([128, KD, HPC * DH], BF16, name="wq_sb")
        wk_sb = const.tile([128, KD, HPC * DH], BF16, name="wk_sb")
        wv_sb = const.tile([128, KD, HPC * DH], BF16, name="wv_sb")
        wo_sb = const.tile([128, HPC, D], BF16, name="wo_sb")
        cc_sb = const.tile([128, NCHUNK, TOKC], BF16, name="cc_sb")  # [cos; cos]
        ss_sb = const.tile([128, NCHUNK, TOKC], BF16, name="ss_sb")  # [-sin; +sin]
        mk_sb = const.tile([128, 128], BF16, name="mk_sb")

        def load_chunk(c):
            xtile = xtp.tile([128, KD, TOKC], BF16, name="xtile")
            src = xt[:, c]  # [128, KD, TOKC], 16KB contiguous per partition
            # two half-loads: matmuls on k<8 start as soon as half 0 lands
            nc.sync.dma_start(xtile[:, 0:KD // 2], src[0:128, 0:KD // 2])
            nc.sync.dma_start(xtile[:, KD // 2:], src[0:128, KD // 2:])
            # RoPE tables for this chunk (contiguous 2KB/partition slices)
            nc.sync.dma_start(cc_sb[:, c], cc[0:128, c * TOKC:(c + 1) * TOKC])
            nc.sync.dma_start(ss_sb[:, c], ss[0:128, c * TOKC:(c + 1) * TOKC])
            return xtile

        # Load order minimizes time-to-first-matmul: the first projection
        # matmuls need only x chunk-0 half a and the wq k<8 half; everything
        # else lands behind them in consumption order (wo last).
        xtile0 = None
        if "qkv" in phases:
            xtile0 = xtp.tile([128, KD, TOKC], BF16, name="xtile")
            nc.sync.dma_start(xtile0[:, 0:KD // 2], xt[:, 0, 0:KD // 2])
        KH = KD // 2
        nc.sync.dma_start(wq_sb[:, 0:KH], wq[:, 0:KH])
        if "qkv" in phases:
            nc.sync.dma_start(xtile0[:, KH:], xt[:, 0, KH:])
            nc.sync.dma_start(cc_sb[:, 0], cc[0:128, 0:TOKC])
            nc.sync.dma_start(ss_sb[:, 0], ss[0:128, 0:TOKC])
        nc.sync.dma_start(wq_sb[:, KH:], wq[:, KH:])
        nc.sync.dma_start(wk_sb[:, 0:KH], wk[:, 0:KH])
        nc.sync.dma_start(wk_sb[:, KH:], wk[:, KH:])
        nc.sync.dma_start(wv_sb[:, 0:KH], wv[:, 0:KH])
        nc.sync.dma_start(wv_sb[:, KH:], wv[:, KH:])
        nc.sync.dma_start(mk_sb, mk)
        nc.sync.dma_start(wo_sb, wo)
        ones_f32 = const.tile([128, 1], F32, name="ones_f32")
        nc.vector.memset(ones_f32, 1.0)
        ones_col = const.tile([128, 1], BF16, name="ones_col")
        nc.vector.tensor_copy(ones_col, ones_f32)

        qrot = [qkvp.tile([128, L], BF16, name=f"qrot{h}") for h in range(HPC)]
        krot = [qkvp.tile([128, L], BF16, name=f"krot{h}") for h in range(HPC)]
        vnat = qkvp.tile([128, KT, HPC * DH], BF16, name="vnat")

        def proj_chunk(c):
            c0 = c * TOKC
            xtile = xtile0 if c == 0 else load_chunk(c)
            cseg = cc_sb[:, c]
            sseg = ss_sb[:, c]
            for w_sb, dsts in ((wq_sb, qrot), (wk_sb, krot)):
                for h in range(HPC):
                    ps = psA.tile([128, TOKC], F32, name="ps_proj", tag="psA")
                    for k in range(KD):
                        nc.tensor.matmul(
                            ps, w_sb[:, k, h * DH:(h + 1) * DH], xtile[:, k, :],
                            start=(k == 0), stop=(k == KD - 1),
                        )
                    if "nocopy" in phases:
                        continue
                    # RoPE copyback: dst = ps*C + swap(ps)*S.  The swap reads
                    # must keep ps in PSUM (partition-shifted SBUF-SBUF reads
                    # are illegal); t/a in bf16 makes the final add take the
                    # DVE double-pumped 16-bit path.
                    t = ropep.tile([128, TOKC], BF16, name="rope_t")
                    a = ropep.tile([128, TOKC], BF16, name="rope_a")
                    nc.vector.tensor_tensor(t[0:64], ps[64:128], sseg[0:64], ALU.mult)
                    nc.vector.tensor_tensor(t[64:128], ps[0:64], sseg[64:128], ALU.mult)
                    nc.vector.tensor_tensor(a, ps, cseg, ALU.mult)
                    nc.vector.tensor_tensor(
                        dsts[h][:, c0:c0 + TOKC], a, t, ALU.add)
            # V in natural [tok, dh] layout: x token slices stationary, all
            # 4 heads share one 512-wide moving pass
            for s in range(TOKC // 128):
                psv = psA.tile([128, HPC * DH], F32, name="ps_v", tag="psA")
                for k in range(KD):
                    nc.tensor.matmul(
                        psv, xtile[:, k, s * 128:(s + 1) * 128], wv_sb[:, k, :],
                        start=(k == 0), stop=(k == KD - 1),
                    )
                if "nocopy" in phases:
                    continue
                kti = (c0 // 128) + s
                # vector, not scalar: the in-order ACT queue must stay clear
                # for the exp chain that gates attnout
                nc.vector.tensor_copy(vnat[:, kti, :], psv)

        def emit_wout(attn_sb, q0):
            # tail-flush version: 16 back-to-back psw groups, double-buffered
            # through psA (scores are done by now), wide [128, D] output rows
            # so stores move 4KB-contiguous packets on the scalar ring
            for mt in range(QC // 128):
                t0 = q0 + mt * 128
                ob = outp.tile([128, D], BF16, name="out_sb")
                for ec in range(D // 512):
                    psw = psA.tile([128, 512], F32, name="ps_w", tag="psA")
                    for h in range(HPC):
                        nc.tensor.matmul(
                            psw, attn_sb[h][:, mt * 128:(mt + 1) * 128],
                            wo_sb[:, h, ec * 512:(ec + 1) * 512],
                            start=(h == 0), stop=(h == HPC - 1),
                        )
                    oseg = ob[:, ec * 512:(ec + 1) * 512]
                    # copybacks alternate scalar/vector, but the last two psw
                    # tiles of the body land on scalar: the next body's first
                    # proj matmuls WAR on these psA slots, and the vector
                    # queue is still busy with attention tail work
                    if (mt + ec) % 2 == 0 or (mt == 3 and ec == 2):
                        nc.scalar.copy(oseg, psw)
                    else:
                        nc.vector.tensor_copy(oseg, psw)
                nc.scalar.dma_start(out[t0:t0 + 128, :], ob)

        def wout_groups(attn_sb, q0):
            # interleaved version: one closure per (mt, ec) psw group, to be
            # consumed one-per-tick inside the NEXT chunk's attention, where
            # the PE is otherwise gated by ACT exp throughput.  Copies on
            # vector (ACT queue stays pure exp), stores on the sync ring
            # (x loads ahead of them have a full chunk of slack).  psW is a
            # dedicated 1-bank pool so the scores psA rotation is untouched.
            obs = [None] * (QC // 128)

            def make(mt, ec):
                def go():
                    if ec == 0:
                        obs[mt] = outp.tile([128, D], BF16, name="out_sb")
                    ob = obs[mt]
                    psw = psW.tile([128, 512], F32, name="ps_wi")
                    for h in range(HPC):
                        nc.tensor.matmul(
                            psw, attn_sb[h][:, mt * 128:(mt + 1) * 128],
                            wo_sb[:, h, ec * 512:(ec + 1) * 512],
                            start=(h == 0), stop=(h == HPC - 1),
                        )
                    nc.vector.tensor_copy(
                        ob[:, ec * 512:(ec + 1) * 512], psw)
                    if ec == D // 512 - 1:
                        t0 = q0 + mt * 128
                        nc.sync.dma_start(out[t0:t0 + 128, :], ob)
                return go

            return [make(mt, ec)
                    for mt in range(QC // 128) for ec in range(D // 512)]

        def attn_chunk(qc, fill=()):
            # `fill` is a list of previous-chunk wout-group closures; one is
            # consumed every `stride` ticks so the ACT-gated attention phase
            # always has independent PE work in the in-order queue
            fill = list(fill)
            total_ticks = HPC * (4 * (qc + 1) // 2 + 4)
            stride = max(1, total_ticks // 16) if fill else 1
            tick = [0]

            def consume_fill():
                if fill and tick[0] % stride == 0:
                    fill.pop(0)()
                tick[0] += 1

            q0 = qc * QC
            attn_sb = []
            for h in range(HPC):
                pso = psO.tile([128, QC], F32, name="ps_out")
                pss = psS.tile([1, QC], F32, name="ps_sum")
                nkt = (qc + 1) * (QC // 128)
                npair = nkt // 2
                nquad = npair // 2
                epairs = [None] * npair
                esums = [None] * npair
                esqs = [None] * nquad

                def emit_attnout(q):
                    e = epairs[q]
                    for half in range(2):
                        kt = 2 * q + half
                        off = 128 * max(kt - 4 * qc, 0)  # causal shrink
                        eh = e[:, half * QC + off:(half + 1) * QC]
                        nc.tensor.matmul(
                            pso[:, off:QC], vnat[:, kt, h * DH:(h + 1) * DH],
                            eh, start=(kt == 0), stop=(kt == nkt - 1))

                for p in range(npair + 4):
                    if p < npair:
                        pp = psA.tile([128, 2 * QC], F32, name="ps_sc", tag="psA")
                        for half in range(2):
                            kt = 2 * p + half
                            off = 128 * max(kt - 4 * qc, 0)  # causal shrink
                            nc.tensor.matmul(
                                pp[:, half * QC + off:(half + 1) * QC],
                                krot[h][:, kt * 128:(kt + 1) * 128],
                                qrot[h][:, q0 + off:q0 + QC],
                                start=True, stop=True,
                            )
                        e = esbp.tile([128, 2 * QC], BF16, name="e_sb")
                        if "noexp" in phases:
                            nc.scalar.copy(e, pp)
                        elif p == npair - 1:
                            # last (diagonal) pair: exp only the live column
                            # ranges (saves ~25% ACT stream on the tile where
                            # ACT throughput gates the attention phase)
                            nc.scalar.activation(
                                e[:, 256:QC], pp[:, 256:QC],
                                AF.Exp, scale=SCALE)
                            nc.scalar.activation(
                                e[:, QC + 384:2 * QC], pp[:, QC + 384:2 * QC],
                                AF.Exp, scale=SCALE)
                        else:
                            nc.scalar.activation(e, pp, AF.Exp, scale=SCALE)
                        if "nomask" not in phases:
                            for half in range(2):
                                if 2 * p + half - 4 * qc >= 0:
                                    off = (half * QC
                                           + 128 * (2 * p + half - 4 * qc))
                                    eh = e[:, off:off + 128]
                                    nc.vector.tensor_tensor(
                                        eh, eh, mk_sb, ALU.mult)
                        epairs[p] = e
                        if "nosum" not in phases:
                            es = esump.tile([128, QC], BF16, name="esum")
                            d0 = 2 * p - 4 * qc
                            if d0 >= 0:  # diagonal pair: halves differ by 128
                                o = 128 * d0
                                nc.vector.tensor_copy(
                                    es[:, o:o + 128], e[:, o:o + 128])
                                nc.vector.tensor_tensor(
                                    es[:, o + 128:QC], e[:, o + 128:QC],
                                    e[:, QC + o + 128:2 * QC], ALU.add)
                            else:
                                nc.vector.tensor_tensor(
                                    es[:, 0:QC], e[:, 0:QC], e[:, QC:2 * QC],
                                    ALU.add)
                            esums[p] = es
                            if p % 2 == 1:
                                # quad presum: fold the two pair-sums into one
                                # tile so the ones-matmul runs once per quad.
                                # The last quad's high pair is diagonal and
                                # valid only from column 256 (its first k-tile
                                # starts 2 blocks past the quad's base).
                                esq = esqp.tile([128, QC], BF16, name="esq")
                                lo, hi = esums[p - 1], esums[p]
                                if p == npair - 1:
                                    nc.vector.tensor_copy(
                                        esq[:, 0:256], lo[:, 0:256])
                                    nc.vector.tensor_tensor(
                                        esq[:, 256:QC], lo[:, 256:QC],
                                        hi[:, 256:QC], ALU.add)
                                else:
                                    nc.vector.tensor_tensor(
                                        esq, lo, hi, ALU.add)
                                esqs[p // 2] = esq
                    # attnout lags 3 ticks behind scores (covers the
                    # exp->mask DVE chain); the quad denominator matmul lags
                    # its esq presum by 3 ticks too, so the in-order PE never
                    # waits on the exp->mask->es->esq chain
                    if 3 <= p < npair + 3:
                        emit_attnout(p - 3)
                    if "nosum" not in phases and p >= 4 and p % 2 == 0:
                        j = (p - 4) // 2
                        if j < nquad:
                            nc.tensor.matmul(
                                pss, ones_col, esqs[j],
                                start=(j == 0), stop=(j == nquad - 1))
                    consume_fill()

                att = attnp.tile([128, QC], BF16, name=f"att{h}")
                if "nosum" in phases:
                    nc.vector.tensor_copy(att, pso)
                else:
                    rec = recp.tile([1, QC], F32, name="recip")
                    nc.vector.reciprocal_approx_fast(rec, pss)
                    bc = bcp.tile([128, QC], F32, name="bc_sb")
                    nc.gpsimd.partition_broadcast(bc, rec)
                    nc.vector.tensor_tensor(att, pso, bc, ALU.mult)
                attn_sb.append(att)
            while fill:  # safety drain (never expected to trigger)
                fill.pop(0)()
            return attn_sb

        pend = pend_in
        for c in range(NCHUNK):
            if "qkv" in phases:
                proj_chunk(c)
            if "attn" in phases:
                fill = (wout_groups(*pend)
                        if pend is not None and "wout" in phases else ())
                attn_sb = attn_chunk(c, fill)
                pend = (attn_sb, c * QC)
        if flush_tail and pend is not None and "wout" in phases:
            emit_wout(*pend)
        return pend


def build_kernel(timing=False, loop_n=0, phases=("qkv", "attn", "wout"),
                 unroll=8):
    nc = bacc.Bacc(
        "TRN2",
        target_bir_lowering=False,
        debug=False,
        enable_asserts=False,
        num_devices=NCORES,
    )
    xt = nc.dram_tensor(
        "xt", [128, NCHUNK, KD, TOKC], BF16, kind="ExternalInput").ap()
    wq = nc.dram_tensor("wq", [128, KD, HPC * DH], BF16, kind="ExternalInput").ap()
    wk = nc.dram_tensor("wk", [128, KD, HPC * DH], BF16, kind="ExternalInput").ap()
    wv = nc.dram_tensor("wv", [128, KD, HPC * DH], BF16, kind="ExternalInput").ap()
    wo = nc.dram_tensor("wo", [128, HPC, D], BF16, kind="ExternalInput").ap()
    cc = nc.dram_tensor("cc", [128, L], BF16, kind="ExternalInput").ap()
    ss = nc.dram_tensor("ss", [128, L], BF16, kind="ExternalInput").ap()
    mk = nc.dram_tensor("mk", [128, 128], BF16, kind="ExternalInput").ap()
    out_kind = "Internal" if timing else "ExternalOutput"
    out = nc.dram_tensor("out", [L, D], BF16, kind=out_kind).ap()
    done = None
    if timing:
        done = nc.dram_tensor("done", [1, 4], BF16, kind="ExternalOutput").ap()

    nc.gpsimd.load_library(library_config.attn)
    aps = (xt, wq, wk, wv, wo, cc, ss, mk, out)
    from contextlib import ExitStack

    with tile.TileContext(nc) as tc, ExitStack() as stk:
        pools = _make_pools(tc, stk)
        if loop_n:
            # Unroll: For_i puts an all-engine barrier + semaphore reset at
            # every loop back-edge, which forbids cross-iteration DMA/compute
            # overlap.  Unrolling the body amortizes that barrier and lets
            # consecutive bodies overlap via normal tile dependency tracking.
            u = unroll if loop_n % unroll == 0 else 1
            with tc.For_i(0, loop_n // u, 1, staggered_reset=True):
                # wout of each body's last chunk is interleaved into the NEXT
                # body's first attention chunk; only the last body before the
                # loop back-edge (an all-engine barrier) flushes it as a block
                pend = None
                for i in range(u):
                    pend = _body(nc, tc, aps, pools, phases,
                                 pend_in=pend, flush_tail=(i == u - 1))
        else:
            _body(nc, tc, aps, pools, phases)
        if timing:
            # tiny output so the executable has an ExternalOutput; depends on
            # one real out tile via a DRAM->DRAM DMA of the last row.
            nc.sync.dma_start(done, out[L - 1:L, 0:4])
    nc.compile()
    return nc


def _rope_tables():
    inv_freq = (1.0 / (ROPE_BASE ** (np.arange(0, DH, 2, dtype=np.float32) / DH))
                ).astype(np.float32)
    freqs = (np.arange(L, dtype=np.float32)[:, None] * inv_freq[None, :]
             ).astype(np.float32)  # [L, 64]
    cos_t = np.cos(freqs).astype(np.float32).T  # [64, L]
    sin_t = np.sin(freqs).astype(np.float32).T
    cc = np.concatenate([cos_t, cos_t], axis=0)  # [128, L]
    ss = np.concatenate([-sin_t, sin_t], axis=0)
    return np.ascontiguousarray(cc), np.ascontiguousarray(ss)


def _host_inputs(x, w_qkv, w_out):
    bf16 = mybir.dt.np(BF16)
    cc, ss = _rope_tables()
    p = np.arange(128)[:, None]
    f = np.arange(128)[None, :]
    mk = np.ascontiguousarray((p <= f).astype(np.float32).astype(bf16))

    def wtile(wT):  # [D, M] f32 -> [128, D//128, M] bf16
        return np.ascontiguousarray(
            wT.reshape(KD, 128, wT.shape[1]).transpose(1, 0, 2)).astype(bf16)

    xts = [
        # [128, NCHUNK, KD, TOKC]: per partition, each chunk is one 16KB
        # contiguous block so the chunk DMA runs at full HBM bandwidth
        np.ascontiguousarray(
            x[b].T.reshape(KD, 128, NCHUNK, TOKC).transpose(1, 2, 0, 3)
        ).astype(bf16)
        for b in range(B)
    ]
    in_maps = []
    for c in range(NCORES):
        b, g = divmod(c, NGRP)
        r0 = g * HPC * DH
        r1 = r0 + HPC * DH
        wq_c = wtile(np.ascontiguousarray(w_qkv[r0:r1, :].T))
        wk_c = wtile(np.ascontiguousarray(w_qkv[D + r0:D + r1, :].T))
        wv_c = wtile(np.ascontiguousarray(w_qkv[2 * D + r0:2 * D + r1, :].T))
        wo_c = np.ascontiguousarray(
            w_out[:, r0:r1].T.reshape(HPC, 128, D).transpose(1, 0, 2)
        ).astype(bf16)
        in_maps.append({
            "xt": xts[b], "wq": wq_c, "wk": wk_c, "wv": wv_c, "wo": wo_c,
            "cc": cc.astype(bf16), "ss": ss.astype(bf16), "mk": mk,
        })
    return in_maps


_NC_CACHE = []


def _get_nc():
    if not _NC_CACHE:
        _NC_CACHE.append(build_kernel())
    return _NC_CACHE[0]


def kernel(x, w_qkv, w_out):
    x = np.asarray(x, dtype=np.float32)
    w_qkv = np.asarray(w_qkv, dtype=np.float32)
    w_out = np.asarray(w_out, dtype=np.float32)
    nc = _get_nc()
    in_maps = _host_inputs(x, w_qkv, w_out)
    res = run_bass_kernel_spmd(nc, in_maps, core_ids=list(range(NCORES)))
    out = np.zeros((B, L, D), dtype=np.float32)
    for c in range(NCORES):
        out[c // NGRP] += res.results[c]["out"].astype(np.float32)
    return out



# revision 54
# speedup vs baseline: 18.6997x; 1.0055x over previous
"""Causal self-attention with RoPE on 8 NeuronCores.

Problem shapes: x [2, 2048, 2048], w_qkv [6144, 2048], w_out [2048, 2048],
H=16 heads, dh=128.

Sharding: data-parallel over batch x tensor-parallel over heads.  Core c
owns batch c//4 and heads {4g..4g+3} with g = c%4.  Each core computes a
[2048, 2048] partial of its batch's output (w_out rows for its heads);
the host sums 4 partials per batch.  Per-core DMA: x 8.4MB bf16 in,
weights 8MB bf16, out 8.4MB bf16 -- about a quarter of the all-TP/f32
layout.

All matmuls run in bf16 (1 row/cycle on PE, f32 PSUM accumulation;
measured rel err ~5e-3 vs the 2e-2 gate).  The PE stream floor for this
decomposition is ~285us/core-iter; the design keeps the matmul pipe >94%
busy by keeping every other engine (and the DMA rings) off its critical
path.  Per-core plan:

  - Q^T/K^T projection: psum[dh, tok] = w-tile (stationary) x xT (moving,
    512-token chunks); V projected into natural [tok, dh] layout by
    flipping operands (x token-slices stationary, wv moving, all 4 heads
    in one 512-wide moving pass).
  - RoPE during PSUM->SBUF copyback, 4 DVE ops per tile using
    sign-folded tables (S = [-sin; +sin], C = [cos; cos], bf16):
    t = swap(ps) * S (2 half ops, PSUM-read partition shift), dst =
    ps * C + t; t/a in bf16 so the add takes the DVE 2x 16-bit path.
  - Attention per 512-token q chunk, k tiles processed in PAIRS sharing
    one [128, 1024] PSUM tile so a single exp activation covers both.
    Attn-out for pair p is emitted after the scores matmuls of pair p+3
    (3-tick lag covers the exp->mask DVE chain on the in-order PE).
  - Causality: diagonal-block matmuls (scores/attn-out) shrink their
    moving free dim to the valid q range; the remaining triangular
    128-col block is masked by a 0/1 multiply (bf16 DVE).  exp runs over
    the full pair tile -- gap regions hold stale PSUM that is never read.
  - Softmax denominator: pair sums e0+e1 (DVE bf16) are folded once more
    into QUAD sums, and a single ones^T x esq matmul per quad accumulates
    the denominator on PE (quarter the ones-matmul stream of per-pair),
    emitted 3 ticks after its esq so the PE never waits on the DVE chain.
    Reciprocal via the approx-fast DVE op (~51 ULP), broadcast across
    partitions with gpsimd, multiplied in during the attn-out copyback.
  - w_out is INTERLEAVED into the next chunk's attention tick stream
    (one 4-matmul psw group per tick, own 1-bank PSUM pool, copybacks on
    vector, stores on the sync ring): within an attention chunk the ACT
    exp throughput (~1.15us/pair) exceeds the PE's own tick work
    (~0.9us/pair), so without filler the in-order PE starves behind the
    exp chain -- the interleave cut measured PE idle from ~21 to ~5
    us/iter (PE ~98% busy).  The final chunk's w_out crosses into the
    NEXT body's first attention chunk (flushed as a block only before
    the loop barrier).  Output rows are assembled into [128, 2048]
    tiles so stores move 4KB-contiguous rows (4KB DMA packets).
  - DMA discipline: all loads issue from the sync engine (ring Q1), all
    stores from the scalar engine (Q10), so loads never queue behind
    stores.  Host-side x layout is [128, chunk, kd, tok] so each chunk
    load is 16KB-contiguous per partition.  Load order minimizes
    time-to-first-matmul (x0 half, wq half first, wo last).
  - The timing build unrolls the body 8x inside tc.For_i: the loop
    back-edge is an all-engine barrier + semaphore reset, so unrolling
    amortizes it and restores cross-body DMA/compute overlap
    (staggered_reset softens the remaining barrier).
"""

import numpy as np

import concourse.bass as bass
import concourse.mybir as mybir
import concourse.tile as tile
from concourse import bacc, library_config
from concourse.bass_utils import run_bass_kernel_spmd

B, L, D, H = 2, 2048, 2048, 16
DH = D // H  # 128
NCORES = 8
NGRP = 4  # head groups (cores per batch)
HPC = H // NGRP  # 4 heads per core
ROPE_BASE = 10000.0
SCALE = 1.0 / float(np.sqrt(np.float32(DH)))

TOKC = 512  # token chunk width in the QKV projection phase
NCHUNK = L // TOKC  # 4
QC = 512  # q chunk width in the attention phase (== TOKC)
KT = L // 128  # 16 k tiles per sequence
KD = D // 128  # 16 contraction chunks for the projections

F32 = mybir.dt.float32
BF16 = mybir.dt.bfloat16
AF = mybir.ActivationFunctionType
ALU = mybir.AluOpType


def _make_pools(tc, stk):
    pool = lambda name, bufs, **kw: stk.enter_context(  # noqa: E731
        tc.tile_pool(name=name, bufs=bufs, **kw))
    return {
        "const": pool("const", 1),
        "xtp": pool("xtp", 2),
        "qkvp": pool("qkv", 1),
        "ropep": pool("rope", 1),
        "esbp": pool("esb", 5),
        "esump": pool("esum", 3),
        "esqp": pool("esq", 3),
        "recp": pool("recp", 1),
        "bcp": pool("bcp", 2),
        "attnp": pool("attn", 2),
        "outp": pool("outp", 4),
        "psA": pool("psA", 2, space="PSUM"),
        "psO": pool("psO", 2, space="PSUM"),
        "psS": pool("psS", 1, space="PSUM"),
        "psW": pool("psW", 1, space="PSUM"),
    }


def _body(nc, tc, aps, P, phases=("qkv", "attn", "wout"), pend_in=None,
          flush_tail=True):
    xt, wq, wk, wv, wo, cc, ss, mk, out = aps
    const, xtp, qkvp, ropep = P["const"], P["xtp"], P["qkvp"], P["ropep"]
    esbp, esump, recp, bcp = P["esbp"], P["esump"], P["recp"], P["bcp"]
    esqp = P["esqp"]
    attnp, outp = P["attnp"], P["outp"]
    psA, psO, psS, psW = P["psA"], P["psO"], P["psS"], P["psW"]
    if True:
        # ---- constants ----
        wq_sb = const.tile([128, KD, HPC * DH], BF16, name="wq_sb")
        wk_sb = const.tile([128, KD, HPC * DH], BF16, name="wk_sb")
        wv_sb = const.tile([128, KD, HPC * DH], BF16, name="wv_sb")
        wo_sb = const.tile([128, HPC, D], BF16, name="wo_sb")
        cc_sb = const.tile([128, NCHUNK, TOKC], BF16, name="cc_sb")  # [cos; cos]
        ss_sb = const.tile([128, NCHUNK, TOKC], BF16, name="ss_sb")  # [-sin; +sin]
        mk_sb = const.tile([128, 128], BF16, name="mk_sb")

        def load_chunk(c):
            xtile = xtp.tile([128, KD, TOKC], BF16, name="xtile")
            src = xt[:, c]  # [128, KD, TOKC], 16KB contiguous per partition
            # two half-loads: matmuls on k<8 start as soon as half 0 lands
            nc.sync.dma_start(xtile[:, 0:KD // 2], src[0:128, 0:KD // 2])
            nc.sync.dma_start(xtile[:, KD // 2:], src[0:128, KD // 2:])
            # RoPE tables for this chunk (contiguous 2KB/partition slices)
            nc.sync.dma_start(cc_sb[:, c], cc[0:128, c * TOKC:(c + 1) * TOKC])
            nc.sync.dma_start(ss_sb[:, c], ss[0:128, c * TOKC:(c + 1) * TOKC])
            return xtile

        # Load order minimizes time-to-first-matmul: the first projection
        # matmuls need only x chunk-0 half a and the wq k<8 half; everything
        # else lands behind them in consumption order (wo last).
        xtile0 = None
        if "qkv" in phases:
            xtile0 = xtp.tile([128, KD, TOKC], BF16, name="xtile")
            nc.sync.dma_start(xtile0[:, 0:KD // 2], xt[:, 0, 0:KD // 2])
        KH = KD // 2
        nc.sync.dma_start(wq_sb[:, 0:KH], wq[:, 0:KH])
        if "qkv" in phases:
            nc.sync.dma_start(xtile0[:, KH:], xt[:, 0, KH:])
            nc.sync.dma_start(cc_sb[:, 0], cc[0:128, 0:TOKC])
            nc.sync.dma_start(ss_sb[:, 0], ss[0:128, 0:TOKC])
        nc.sync.dma_start(wq_sb[:, KH:], wq[:, KH:])
        nc.sync.dma_start(wk_sb[:, 0:KH], wk[:, 0:KH])
        nc.sync.dma_start(wk_sb[:, KH:], wk[:, KH:])
        nc.sync.dma_start(wv_sb[:, 0:KH], wv[:, 0:KH])
        nc.sync.dma_start(wv_sb[:, KH:], wv[:, KH:])
        nc.sync.dma_start(mk_sb, mk)
        nc.sync.dma_start(wo_sb, wo)
        ones_f32 = const.tile([128, 1], F32, name="ones_f32")
        nc.vector.memset(ones_f32, 1.0)
        ones_col = const.tile([128, 1], BF16, name="ones_col")
        nc.vector.tensor_copy(ones_col, ones_f32)

        qrot = [qkvp.tile([128, L], BF16, name=f"qrot{h}") for h in range(HPC)]
        krot = [qkvp.tile([128, L], BF16, name=f"krot{h}") for h in range(HPC)]
        vnat = qkvp.tile([128, KT, HPC * DH], BF16, name="vnat")

        def proj_chunk(c):
            c0 = c * TOKC
            xtile = xtile0 if c == 0 else load_chunk(c)
            cseg = cc_sb[:, c]
            sseg = ss_sb[:, c]
            for w_sb, dsts in ((wq_sb, qrot), (wk_sb, krot)):
                for h in range(HPC):
                    ps = psA.tile([128, TOKC], F32, name="ps_proj", tag="psA")
                    for k in range(KD):
                        nc.tensor.matmul(
                            ps, w_sb[:, k, h * DH:(h + 1) * DH], xtile[:, k, :],
                            start=(k == 0), stop=(k == KD - 1),
                        )
                    if "nocopy" in phases:
                        continue
                    # RoPE copyback: dst = ps*C + swap(ps)*S.  The swap reads
                    # must keep ps in PSUM (partition-shifted SBUF-SBUF reads
                    # are illegal); t/a in bf16 makes the final add take the
                    # DVE double-pumped 16-bit path.
                    t = ropep.tile([128, TOKC], BF16, name="rope_t")
                    a = ropep.tile([128, TOKC], BF16, name="rope_a")
                    nc.vector.tensor_tensor(t[0:64], ps[64:128], sseg[0:64], ALU.mult)
                    nc.vector.tensor_tensor(t[64:128], ps[0:64], sseg[64:128], ALU.mult)
                    nc.vector.tensor_tensor(a, ps, cseg, ALU.mult)
                    nc.vector.tensor_tensor(
                        dsts[h][:, c0:c0 + TOKC], a, t, ALU.add)
            # V in natural [tok, dh] layout: x token slices stationary, all
            # 4 heads share one 512-wide moving pass
            for s in range(TOKC // 128):
                psv = psA.tile([128, HPC * DH], F32, name="ps_v", tag="psA")
                for k in range(KD):
                    nc.tensor.matmul(
                        psv, xtile[:, k, s * 128:(s + 1) * 128], wv_sb[:, k, :],
                        start=(k == 0), stop=(k == KD - 1),
                    )
                if "nocopy" in phases:
                    continue
                kti = (c0 // 128) + s
                # vector, not scalar: the in-order ACT queue must stay clear
                # for the exp chain that gates attnout
                nc.vector.tensor_copy(vnat[:, kti, :], psv)

        def emit_wout(attn_sb, q0):
            # tail-flush version: 16 back-to-back psw groups, double-buffered
            # through psA (scores are done by now), wide [128, D] output rows
            # so stores move 4KB-contiguous packets on the scalar ring
            for mt in range(QC // 128):
                t0 = q0 + mt * 128
                ob = outp.tile([128, D], BF16, name="out_sb")
                for ec in range(D // 512):
                    psw = psA.tile([128, 512], F32, name="ps_w", tag="psA")
                    for h in range(HPC):
                        nc.tensor.matmul(
                            psw, attn_sb[h][:, mt * 128:(mt + 1) * 128],
                            wo_sb[:, h, ec * 512:(ec + 1) * 512],
                            start=(h == 0), stop=(h == HPC - 1),
                        )
                    oseg = ob[:, ec * 512:(ec + 1) * 512]
                    # copybacks alternate scalar/vector, but the last two psw
                    # tiles of the body land on scalar: the next body's first
                    # proj matmuls WAR on these psA slots, and the vector
                    # queue is still busy with attention tail work
                    if (mt + ec) % 2 == 0 or (mt == 3 and ec == 2):
                        nc.scalar.copy(oseg, psw)
                    else:
                        nc.vector.tensor_copy(oseg, psw)
                nc.scalar.dma_start(out[t0:t0 + 128, :], ob)

        def wout_groups(attn_sb, q0):
            # interleaved version: one closure per (mt, ec) psw group, to be
            # consumed one-per-tick inside the NEXT chunk's attention, where
            # the PE is otherwise gated by ACT exp throughput.  Copies on
            # vector (ACT queue stays pure exp), stores on the sync ring
            # (x loads ahead of them have a full chunk of slack).  psW is a
            # dedicated 1-bank pool so the scores psA rotation is untouched.
            obs = [None] * (QC // 128)

            def make(mt, ec):
                def go():
                    if ec == 0:
                        obs[mt] = outp.tile([128, D], BF16, name="out_sb")
                    ob = obs[mt]
                    psw = psW.tile([128, 512], F32, name="ps_wi")
                    for h in range(HPC):
                        nc.tensor.matmul(
                            psw, attn_sb[h][:, mt * 128:(mt + 1) * 128],
                            wo_sb[:, h, ec * 512:(ec + 1) * 512],
                            start=(h == 0), stop=(h == HPC - 1),
                        )
                    nc.vector.tensor_copy(
                        ob[:, ec * 512:(ec + 1) * 512], psw)
                    if ec == D // 512 - 1:
                        t0 = q0 + mt * 128
                        nc.sync.dma_start(out[t0:t0 + 128, :], ob)
                return go

            return [make(mt, ec)
                    for mt in range(QC // 128) for ec in range(D // 512)]

        def attn_chunk(qc, fill=()):
            # `fill` is a list of previous-chunk wout-group closures; one is
            # consumed every `stride` ticks so the ACT-gated attention phase
            # always has independent PE work in the in-order queue
            fill = list(fill)
            total_ticks = HPC * (4 * (qc + 1) // 2 + 4)
            stride = max(1, total_ticks // 16) if fill else 1
            tick = [0]

            def consume_fill():
                if fill and tick[0] % stride == 0:
                    fill.pop(0)()
                tick[0] += 1

            q0 = qc * QC
            attn_sb = []
            for h in range(HPC):
                pso = psO.tile([128, QC], F32, name="ps_out")
                pss = psS.tile([1, QC], F32, name="ps_sum")
                nkt = (qc + 1) * (QC // 128)
                npair = nkt // 2
                nquad = npair // 2
                epairs = [None] * npair
                esums = [None] * npair
                esqs = [None] * nquad

                def emit_attnout(q):
                    e = epairs[q]
                    for half in range(2):
                        kt = 2 * q + half
                        off = 128 * max(kt - 4 * qc, 0)  # causal shrink
                        eh = e[:, half * QC + off:(half + 1) * QC]
                        nc.tensor.matmul(
                            pso[:, off:QC], vnat[:, kt, h * DH:(h + 1) * DH],
                            eh, start=(kt == 0), stop=(kt == nkt - 1))

                for p in range(npair + 4):
                    if p < npair:
                        pp = psA.tile([128, 2 * QC], F32, name="ps_sc", tag="psA")
                        for half in range(2):
                            kt = 2 * p + half
                            off = 128 * max(kt - 4 * qc, 0)  # causal shrink
                            nc.tensor.matmul(
                                pp[:, half * QC + off:(half + 1) * QC],
                                krot[h][:, kt * 128:(kt + 1) * 128],
                                qrot[h][:, q0 + off:q0 + QC],
                                start=True, stop=True,
                            )
                        e = esbp.tile([128, 2 * QC], BF16, name="e_sb")
                        if "noexp" in phases:
                            nc.scalar.copy(e, pp)
                        elif p == npair - 1:
                            # last (diagonal) pair: exp only the live column
                            # ranges (saves ~25% ACT stream on the tile where
                            # ACT throughput gates the attention phase)
                            nc.scalar.activation(
                                e[:, 256:QC], pp[:, 256:QC],
                                AF.Exp, scale=SCALE)
                            nc.scalar.activation(
                                e[:, QC + 384:2 * QC], pp[:, QC + 384:2 * QC],
                                AF.Exp, scale=SCALE)
                        else:
                            nc.scalar.activation(e, pp, AF.Exp, scale=SCALE)
                        if "nomask" not in phases:
                            for half in range(2):
                                if 2 * p + half - 4 * qc >= 0:
                                    off = (half * QC
                                           + 128 * (2 * p + half - 4 * qc))
                                    eh = e[:, off:off + 128]
                                    nc.vector.tensor_tensor(
                                        eh, eh, mk_sb, ALU.mult)
                        epairs[p] = e
                        if "nosum" not in phases:
                            es = esump.tile([128, QC], BF16, name="esum")
                            d0 = 2 * p - 4 * qc
                            if d0 >= 0:  # diagonal pair: halves differ by 128
                                o = 128 * d0
                                nc.vector.tensor_copy(
                                    es[:, o:o + 128], e[:, o:o + 128])
                                nc.vector.tensor_tensor(
                                    es[:, o + 128:QC], e[:, o + 128:QC],
                                    e[:, QC + o + 128:2 * QC], ALU.add)
                            else:
                                nc.vector.tensor_tensor(
                                    es[:, 0:QC], e[:, 0:QC], e[:, QC:2 * QC],
                                    ALU.add)
                            esums[p] = es
                            if p % 2 == 1:
                                # quad presum: fold the two pair-sums into one
                                # tile so the ones-matmul runs once per quad.
                                # The last quad's high pair is diagonal and
                                # valid only from column 256 (its first k-tile
                                # starts 2 blocks past the quad's base).
                                esq = esqp.tile([128, QC], BF16, name="esq")
                                lo, hi = esums[p - 1], esums[p]
                                if p == npair - 1:
                                    nc.vector.tensor_copy(
                                        esq[:, 0:256], lo[:, 0:256])
                                    nc.vector.tensor_tensor(
                                        esq[:, 256:QC], lo[:, 256:QC],
                                        hi[:, 256:QC], ALU.add)
                                else:
                                    nc.vector.tensor_tensor(
                                        esq, lo, hi, ALU.add)
                                esqs[p // 2] = esq
                    # attnout lags 3 ticks behind scores (covers the
                    # exp->mask DVE chain); the quad denominator matmul lags
                    # its esq presum by 3 ticks too, so the in-order PE never
                    # waits on the exp->mask->es->esq chain
                    if 3 <= p < npair + 3:
                        emit_attnout(p - 3)
                    if "nosum" not in phases and p >= 4 and p % 2 == 0:
                        j = (p - 4) // 2
                        if j < nquad:
                            nc.tensor.matmul(
                                pss, ones_col, esqs[j],
                                start=(j == 0), stop=(j == nquad - 1))
                    consume_fill()

                att = attnp.tile([128, QC], BF16, name=f"att{h}")
                if "nosum" in phases:
                    nc.vector.tensor_copy(att, pso)
                else:
                    rec = recp.tile([1, QC], F32, name="recip")
                    nc.vector.reciprocal_approx_fast(rec, pss)
                    bc = bcp.tile([128, QC], F32, name="bc_sb")
                    nc.gpsimd.partition_broadcast(bc, rec)
                    nc.vector.tensor_tensor(att, pso, bc, ALU.mult)
                attn_sb.append(att)
            while fill:  # safety drain (never expected to trigger)
                fill.pop(0)()
            return attn_sb

        pend = pend_in
        for c in range(NCHUNK):
            if "qkv" in phases:
                proj_chunk(c)
            if "attn" in phases:
                fill = (wout_groups(*pend)
                        if pend is not None and "wout" in phases else ())
                attn_sb = attn_chunk(c, fill)
                pend = (attn_sb, c * QC)
        if flush_tail and pend is not None and "wout" in phases:
            emit_wout(*pend)
        return pend


def build_kernel(timing=False, loop_n=0, phases=("qkv", "attn", "wout"),
                 unroll=16):
    nc = bacc.Bacc(
        "TRN2",
        target_bir_lowering=False,
        debug=False,
        enable_asserts=False,
        num_devices=NCORES,
    )
    xt = nc.dram_tensor(
        "xt", [128, NCHUNK, KD, TOKC], BF16, kind="ExternalInput").ap()
    wq = nc.dram_tensor("wq", [128, KD, HPC * DH], BF16, kind="ExternalInput").ap()
    wk = nc.dram_tensor("wk", [128, KD, HPC * DH], BF16, kind="ExternalInput").ap()
    wv = nc.dram_tensor("wv", [128, KD, HPC * DH], BF16, kind="ExternalInput").ap()
    wo = nc.dram_tensor("wo", [128, HPC, D], BF16, kind="ExternalInput").ap()
    cc = nc.dram_tensor("cc", [128, L], BF16, kind="ExternalInput").ap()
    ss = nc.dram_tensor("ss", [128, L], BF16, kind="ExternalInput").ap()
    mk = nc.dram_tensor("mk", [128, 128], BF16, kind="ExternalInput").ap()
    out_kind = "Internal" if timing else "ExternalOutput"
    out = nc.dram_tensor("out", [L, D], BF16, kind=out_kind).ap()
    done = None
    if timing:
        done = nc.dram_tensor("done", [1, 4], BF16, kind="ExternalOutput").ap()

    nc.gpsimd.load_library(library_config.attn)
    aps = (xt, wq, wk, wv, wo, cc, ss, mk, out)
    from contextlib import ExitStack

    with tile.TileContext(nc) as tc, ExitStack() as stk:
        pools = _make_pools(tc, stk)
        if loop_n:
            # Unroll: For_i puts an all-engine barrier + semaphore reset at
            # every loop back-edge, which forbids cross-iteration DMA/compute
            # overlap.  Unrolling the body amortizes that barrier and lets
            # consecutive bodies overlap via normal tile dependency tracking.
            u = unroll if loop_n % unroll == 0 else 1
            with tc.For_i(0, loop_n // u, 1, staggered_reset=True):
                # wout of each body's last chunk is interleaved into the NEXT
                # body's first attention chunk; only the last body before the
                # loop back-edge (an all-engine barrier) flushes it as a block
                pend = None
                for i in range(u):
                    pend = _body(nc, tc, aps, pools, phases,
                                 pend_in=pend, flush_tail=(i == u - 1))
        else:
            _body(nc, tc, aps, pools, phases)
        if timing:
            # tiny output so the executable has an ExternalOutput; depends on
            # one real out tile via a DRAM->DRAM DMA of the last row.
            nc.sync.dma_start(done, out[L - 1:L, 0:4])
    nc.compile()
    return nc


def _rope_tables():
    inv_freq = (1.0 / (ROPE_BASE ** (np.arange(0, DH, 2, dtype=np.float32) / DH))
                ).astype(np.float32)
    freqs = (np.arange(L, dtype=np.float32)[:, None] * inv_freq[None, :]
             ).astype(np.float32)  # [L, 64]
    cos_t = np.cos(freqs).astype(np.float32).T  # [64, L]
    sin_t = np.sin(freqs).astype(np.float32).T
    cc = np.concatenate([cos_t, cos_t], axis=0)  # [128, L]
    ss = np.concatenate([-sin_t, sin_t], axis=0)
    return np.ascontiguousarray(cc), np.ascontiguousarray(ss)


def _host_inputs(x, w_qkv, w_out):
    bf16 = mybir.dt.np(BF16)
    cc, ss = _rope_tables()
    p = np.arange(128)[:, None]
    f = np.arange(128)[None, :]
    mk = np.ascontiguousarray((p <= f).astype(np.float32).astype(bf16))

    def wtile(wT):  # [D, M] f32 -> [128, D//128, M] bf16
        return np.ascontiguousarray(
            wT.reshape(KD, 128, wT.shape[1]).transpose(1, 0, 2)).astype(bf16)

    xts = [
        # [128, NCHUNK, KD, TOKC]: per partition, each chunk is one 16KB
        # contiguous block so the chunk DMA runs at full HBM bandwidth
        np.ascontiguousarray(
            x[b].T.reshape(KD, 128, NCHUNK, TOKC).transpose(1, 2, 0, 3)
        ).astype(bf16)
        for b in range(B)
    ]
    in_maps = []
    for c in range(NCORES):
        b, g = divmod(c, NGRP)
        r0 = g * HPC * DH
        r1 = r0 + HPC * DH
        wq_c = wtile(np.ascontiguousarray(w_qkv[r0:r1, :].T))
        wk_c = wtile(np.ascontiguousarray(w_qkv[D + r0:D + r1, :].T))
        wv_c = wtile(np.ascontiguousarray(w_qkv[2 * D + r0:2 * D + r1, :].T))
        wo_c = np.ascontiguousarray(
            w_out[:, r0:r1].T.reshape(HPC, 128, D).transpose(1, 0, 2)
        ).astype(bf16)
        in_maps.append({
            "xt": xts[b], "wq": wq_c, "wk": wk_c, "wv": wv_c, "wo": wo_c,
            "cc": cc.astype(bf16), "ss": ss.astype(bf16), "mk": mk,
        })
    return in_maps


_NC_CACHE = []


def _get_nc():
    if not _NC_CACHE:
        _NC_CACHE.append(build_kernel())
    return _NC_CACHE[0]


def kernel(x, w_qkv, w_out):
    x = np.asarray(x, dtype=np.float32)
    w_qkv = np.asarray(w_qkv, dtype=np.float32)
    w_out = np.asarray(w_out, dtype=np.float32)
    nc = _get_nc()
    in_maps = _host_inputs(x, w_qkv, w_out)
    res = run_bass_kernel_spmd(nc, in_maps, core_ids=list(range(NCORES)))
    out = np.zeros((B, L, D), dtype=np.float32)
    for c in range(NCORES):
        out[c // NGRP] += res.results[c]["out"].astype(np.float32)
    return out

